# revision 1
# baseline (speedup 1.0000x reference)
"""Trainium2 Bass kernel for NEAT-style genome evaluation (gnn_message_passing).

Shapes are hardcoded for the problem:
  inputs [16384, 256] f32, in_idx/edge_w/edge_mask [768, 8], bias/response [768],
  out_idx [768] (scan order), output_idx [64]; output [16384, 64] f32.

Strategy: pure data-parallel over batch (2048 rows/core on 8 cores). Per core,
node values are stored node-major in SBUF ([node, batch]) in fp16, nodes ordered
by topological level and packed into 128-row chunks aligned to level boundaries.
Each chunk's pre-activations accumulate in PSUM via fp16 matmuls whose weight
matrices embed the (sparse) DAG edges; cross-chunk contributions stream once per
(dst-chunk, src-chunk) pair, intra-chunk contributions once per level.
tanh(bias + response*s) runs on the scalar engine per (level, batch-half).
Cross-chunk matmuls of the NEXT chunk are interleaved into the current chunk's
level cascade so the tensor engine fills the gaps left by the activation chain.
Input marshaling (batch-major -> node-major) uses fp16 PE transposes with the
f32->fp16 conversion on the Pool engine; the 64 output nodes are gathered with
one-hot matmuls. fp16 keeps end-to-end relative error ~3e-3 (tolerance 2e-2).
"""

import sys

import numpy as np

if "/opt/trn_rl_repo" not in sys.path:
    sys.path.insert(0, "/opt/trn_rl_repo")

import concourse.bacc as bacc
import concourse.mybir as mybir
from concourse.bass_utils import run_bass_kernel_spmd
from concourse.masks import make_identity
from concourse.tile import TileContext

F16 = mybir.dt.float16
F32 = mybir.dt.float32

B = 16384
NUM_IN = 256
N = 1024
K = 8
NN = N - NUM_IN
NUM_OUT = 64
NCORES = 8
BC = B // NCORES          # batch rows per core
NBT = BC // 128           # batch tiles of 128 per core
HALF = BC // 2            # free columns per half-chain
NSUB = HALF // 512        # 512-wide matmul sub-blocks per half


def _plan(in_idx, edge_mask, edge_w, bias, response, out_idx, output_idx):
    """All host-side graph analysis; returns the constant tensors + schedule."""
    in_idx = np.asarray(in_idx)
    edge_mask = np.asarray(edge_mask).astype(bool)
    edge_w = np.asarray(edge_w).astype(np.float32)
    bias = np.asarray(bias).astype(np.float32)
    response = np.asarray(response).astype(np.float32)
    out_idx = np.asarray(out_idx)
    output_idx = np.asarray(output_idx)

    # scan write position of each node (reference writes out_idx[r] at step r)
    write_pos = np.full(N, -1, dtype=np.int64)
    for r in range(NN):
        write_pos[out_idx[r]] = r

    # valid edges: mask set AND source reads a value written before this step
    valid = np.zeros((NN, K), dtype=bool)
    for r in range(NN):
        for k in range(K):
            if not edge_mask[r, k]:
                continue
            s = int(in_idx[r, k])
            if s < NUM_IN or (0 <= write_pos[s] < r):
                valid[r, k] = True

    # prune nodes that do not reach any output
    needed = np.zeros(N, dtype=bool)
    needed[output_idx] = True
    for r in range(NN - 1, -1, -1):
        d = out_idx[r]
        if needed[d]:
            for k in range(K):
                if valid[r, k]:
                    needed[in_idx[r, k]] = True

    # topological levels over reachable non-input nodes (inputs = level 0)
    level = np.zeros(N, dtype=np.int64)
    for r in range(NN):
        d = out_idx[r]
        if not needed[d]:
            continue
        lmax = 0
        for k in range(K):
            if valid[r, k]:
                lmax = max(lmax, level[in_idx[r, k]] + 1)
        level[d] = lmax
    depth = int(level[needed].max()) if needed.any() else 0

    # split any level wider than 128 (keeps chunk packing valid)
    groups = []  # arrays of node ids, dependency order
    for l in range(1, depth + 1):
        nodes = [out_idx[r] for r in range(NN)
                 if needed[out_idx[r]] and level[out_idx[r]] == l]
        nodes = np.array(sorted(nodes, key=lambda d: write_pos[d]), dtype=np.int64)
        for i in range(0, len(nodes), 128):
            groups.append(nodes[i:i + 128])

    # pack whole groups into 128-row node chunks
    chunks = []   # list of list[(group_nodes, local_start)]
    fill = 128
    for g in groups:
        if fill + len(g) > 128:
            chunks.append([])
            fill = 0
        chunks[-1].append((g, fill))
        fill += len(g)

    n_in_chunks = NUM_IN // 128          # 2
    n_node_chunks = len(chunks)
    n_chunks = n_in_chunks + n_node_chunks

    chunk_of = np.full(N, -1, dtype=np.int64)
    row_of = np.full(N, -1, dtype=np.int64)
    for j in range(NUM_IN):
        chunk_of[j] = j // 128
        row_of[j] = j % 128
    for ci, levs in enumerate(chunks):
        for g, start in levs:
            for i, d in enumerate(g):
                chunk_of[d] = n_in_chunks + ci
                row_of[d] = start + i

    # per-node bias/response laid out per chunk
    bias_c = np.zeros((128, n_node_chunks), dtype=np.float32)
    resp_c = np.ones((128, n_node_chunks), dtype=np.float32)
    for r in range(NN):
        d = out_idx[r]
        if not needed[d]:
            continue
        bias_c[row_of[d], chunk_of[d] - n_in_chunks] = bias[r]
        resp_c[row_of[d], chunk_of[d] - n_in_chunks] = response[r]

    # weight blocks
    wa_blocks = {}   # (dst_chunk_rel, src_chunk_abs) -> [128,128]
    wb_blocks = {}   # (dst_chunk_rel, group_idx_in_chunk) -> [128,128]
    for r in range(NN):
        d = out_idx[r]
        if not needed[d]:
            continue
        dc = chunk_of[d] - n_in_chunks
        for k in range(K):
            if not valid[r, k]:
                continue
            s = int(in_idx[r, k])
            w = float(edge_w[r, k])
            sc = chunk_of[s]
            if sc == chunk_of[d]:
                gi = next(i for i, (g, st) in enumerate(chunks[dc])
                          if st <= row_of[d] < st + len(g))
                blk = wb_blocks.setdefault((dc, gi), np.zeros((128, 128), np.float32))
            else:
                blk = wa_blocks.setdefault((dc, sc), np.zeros((128, 128), np.float32))
            blk[row_of[s], row_of[d]] += w

    parta = []   # per node chunk: list of (src_chunk, wa_index)
    wa_list = []
    for dc in range(n_node_chunks):
        lst = []
        for sc in range(n_chunks):
            if (dc, sc) in wa_blocks:
                lst.append((sc, len(wa_list)))
                wa_list.append(wa_blocks[(dc, sc)])
        parta.append(lst)

    partb = []   # per node chunk: list of (group_idx, local_start, m, wb_index|None)
    wb_list = []
    for dc in range(n_node_chunks):
        lst = []
        for gi, (g, st) in enumerate(chunks[dc]):
            if (dc, gi) in wb_blocks:
                lst.append((gi, st, len(g), len(wb_list)))
                wb_list.append(wb_blocks[(dc, gi)])
            else:
                lst.append((gi, st, len(g), None))
        partb.append(lst)

    # output extraction: row-ranges per chunk covering its output nodes
    # (split at gaps of non-output rows; the host selects columns at the end)
    rows_by_chunk = {}
    for d in output_idx:
        dc = int(chunk_of[d]) - n_in_chunks
        rows_by_chunk.setdefault(dc, []).append(int(row_of[d]))
    out_ranges = []   # (dst_chunk_rel, row0, len, col0)
    col = 0
    pos_of = {}       # (dc, row) -> staged column
    last_oc = max(rows_by_chunk)
    for dc in sorted(rows_by_chunk):
        rows = sorted(set(rows_by_chunk[dc]))
        gap = 10**9 if dc == last_oc else 8
        start = prev = rows[0]
        for r in rows[1:] + [None]:
            if r is not None and r - prev <= gap:
                prev = r
                continue
            ln = prev - start + 1
            out_ranges.append((dc, start, ln, col))
            for rr in range(start, prev + 1):
                pos_of[(dc, rr)] = col + rr - start
            col += ln
            if r is not None:
                start = prev = r
    ntot = col
    assert ntot <= 192, f"staged output rows {ntot} too large"
    colmap = np.zeros(NUM_OUT, dtype=np.int64)
    for oc, d in enumerate(output_idx):
        dc = int(chunk_of[d]) - n_in_chunks
        colmap[oc] = pos_of[(dc, int(row_of[d]))]

    return dict(
        out_ranges=out_ranges,
        ntot=ntot,
        colmap=colmap,
        n_in_chunks=n_in_chunks,
        n_node_chunks=n_node_chunks,
        n_chunks=n_chunks,
        parta=parta,
        partb=partb,
        wa=(np.stack(wa_list) if wa_list
            else np.zeros((0, 128, 128), np.float32)).astype(np.float16),
        wb=(np.stack(wb_list) if wb_list
            else np.zeros((0, 128, 128), np.float32)).astype(np.float16),
        bias_c=bias_c,
        resp_c=resp_c,
    )


def _build_nc(plan):
    n_in_chunks = plan["n_in_chunks"]
    n_node_chunks = plan["n_node_chunks"]
    n_chunks = plan["n_chunks"]
    parta = plan["parta"]
    partb = plan["partb"]
    out_ranges = plan["out_ranges"]
    ntot = plan["ntot"]
    n_wa = len(plan["wa"])
    n_wb = len(plan["wb"])

    nc = bacc.Bacc()
    x = nc.dram_tensor("x", [BC, NUM_IN], F16, kind="ExternalInput")
    wa = nc.dram_tensor("wa", [max(n_wa, 1), 128, 128], F16, kind="ExternalInput")
    wb = nc.dram_tensor("wb", [max(n_wb, 1), 128, 128], F16, kind="ExternalInput")
    br_d = nc.dram_tensor("br_c", [128, 2 * n_node_chunks], F32, kind="ExternalInput")
    o = nc.dram_tensor("o", [BC, ntot], F32, kind="ExternalOutput")

    with TileContext(nc) as tc:
        with tc.tile_pool(name="const", bufs=1) as const, \
             tc.tile_pool(name="vpool", bufs=2 * n_chunks) as vpool, \
             tc.tile_pool(name="bmpool", bufs=1) as bmpool, \
             tc.tile_pool(name="obpool", bufs=1) as obpool, \
             tc.tile_pool(name="pc", bufs=3, space="PSUM") as pcp, \
             tc.tile_pool(name="mt", bufs=2, space="PSUM") as mtp:

            ident = const.tile([128, 128], F16)
            make_identity(nc, ident[:])
            br_sb = const.tile([128, 2 * n_node_chunks], F32, tag="br")
            bias_sb = br_sb[:, 0:n_node_chunks]
            resp_sb = br_sb[:, n_node_chunks:2 * n_node_chunks]

            wa_sb = const.tile([128, max(n_wa, 1) * 128], F16, tag="wa_sb")
            wb_sb = const.tile([128, max(n_wb, 1) * 128], F16, tag="wb_sb")

            # node-major value store: v[chunk][half] = [128, HALF] fp16
            v = [[vpool.tile([128, HALF], F16, tag="v", name=f"v{c}h{h}")
                  for h in range(2)]
                 for c in range(n_chunks)]

            bm = bmpool.tile([128, NBT, NUM_IN], F16, tag="bm")
            ob = obpool.tile([128, NBT, ntot], F32)
            xr = x.rearrange("(t p) f -> p t f", p=128)
            orr = o.rearrange("(t p) f -> p t f", p=128)

            # ---- DMA schedule, urgency-ordered
            nc.sync.dma_start(bm[:, 0:2, :], xr[:, 0:2, :])
            nc.sync.dma_start(bm[:, 2:4, :], xr[:, 2:4, :])
            nc.sync.dma_start(bm[:, 4:8, :], xr[:, 4:8, :])
            nc.scalar.dma_start(br_sb[:], br_d[:])

            def wa_span(dc):
                idxs = [ai for _, ai in parta[dc]]
                return (idxs[0], idxs[-1] + 1) if idxs else None

            def wb_span(dc):
                bidx = [bi for _, _, _, bi in partb[dc] if bi is not None]
                return (bidx[0], bidx[-1] + 1) if bidx else None

            def dma_wa(i0, i1):
                nc.sync.dma_start(wa_sb[:, i0 * 128:i1 * 128],
                                  wa[i0:i1].rearrange("n p f -> p n f"))

            def dma_wb(i0, i1):
                nc.sync.dma_start(wb_sb[:, i0 * 128:i1 * 128],
                                  wb[i0:i1].rearrange("n p f -> p n f"))

            s = wa_span(0)
            if s:
                dma_wa(*s)
            s = wb_span(0)
            if s:
                dma_wb(*s)
            # second input half next -- gates the h1 marshal
            nc.sync.dma_start(bm[:, 8:12, :], xr[:, 8:12, :])
            nc.sync.dma_start(bm[:, 12:16, :], xr[:, 12:16, :])
            s01 = wa_span(1)
            if s01:
                dma_wa(*s01)
            s = wb_span(1)
            if s:
                dma_wb(*s)
            a_lo = wa_span(2)[0] if n_node_chunks > 2 and wa_span(2) else n_wa
            if a_lo < n_wa:
                mid = (a_lo + n_wa + 1) // 2
                dma_wa(a_lo, mid)
                dma_wa(mid, n_wa)
            b_lo = wb_span(2)[0] if n_node_chunks > 2 and wb_span(2) else n_wb
            if b_lo < n_wb:
                dma_wb(b_lo, n_wb)

            # ---- input marshal: fp16 PE transpose, DVE/Pool copy to v store;
            # emitted per half so chunk 0 h0 can start before h1's DMA lands
            def marshal(h):
                t0 = h * (NBT // 2)
                for sub in range(NSUB):
                    for cin in range(n_in_chunks):
                        mt = mtp.tile([128, 512], F16, tag="mt",
                                      name=f"mt_h{h}s{sub}c{cin}")
                        for q in range(4):
                            t = t0 + sub * 4 + q
                            nc.tensor.transpose(
                                mt[:, q * 128:(q + 1) * 128],
                                bm[:, t, cin * 128:(cin + 1) * 128],
                                ident[:])
                        nc.vector.tensor_copy(
                            v[cin][h][:, sub * 512:(sub + 1) * 512], mt[:])

            # ---- weight cascade with per-half psum tiles and cross-chunk
            # interleave of the next chunk's parta blocks
            pc_tiles = [[None, None] for _ in range(n_node_chunks)]
            parta_done = [[0, 0] for _ in range(n_node_chunks)]

            def emit_parta_one(dc, idx, h):
                sc, ai = parta[dc][idx]
                if pc_tiles[dc][h] is None:
                    pc_tiles[dc][h] = pcp.tile([128, HALF], F32, tag="pc",
                                               name=f"pc{dc}h{h}")
                wt = wa_sb[:, ai * 128:(ai + 1) * 128]
                first = (parta_done[dc][h] == 0)
                pc = pc_tiles[dc][h]
                for sub in range(NSUB):
                    nc.tensor.matmul(
                        pc[:, sub * 512:(sub + 1) * 512],
                        wt,
                        v[sc][h][:, sub * 512:(sub + 1) * 512],
                        start=first, stop=False,
                        skip_group_check=True)
                parta_done[dc][h] += 1

            # prologue: marshal + chunk-0 parta for h0; h1's marshal and parta
            # are deferred into chunk 0's first level so the h0 chain is not
            # queued behind the h1 input DMA on the in-order tensor engine
            marshal(0)
            for i in range(len(parta[0])):
                emit_parta_one(0, i, 0)

            pending_h1 = {0: list(range(len(parta[0])))}
            marshal_h1_pending = [True]
            for dc in range(n_node_chunks):
                gc = n_in_chunks + dc
                wbts = {gi: wb_sb[:, bi * 128:(bi + 1) * 128]
                        for gi, st, m, bi in partb[dc] if bi is not None}
                last_gi = max(gi for gi, _, _, _ in partb[dc])

                # next chunk's parta: blocks not reading chunk gc interleave
                # into this chunk's levels (h0 only; h1 psum frees later);
                # blocks reading gc and all h1 blocks go after the last ACT
                if dc + 1 < n_node_chunks:
                    nxt = [i for i, (sc, _) in enumerate(parta[dc + 1])
                           if sc < gc]
                    nxt_late = [i for i, (sc, _) in enumerate(parta[dc + 1])
                                if sc >= gc]
                else:
                    nxt, nxt_late = [], []
                ngroups = len(partb[dc])
                per = min(1, -(-len(nxt) // ngroups)) if nxt else 0

                for idx, (gi, st, m, bi) in enumerate(partb[dc]):
                    for h in range(2):
                        if h == 1 and idx == 0:
                            # this chunk's deferred h1 cross-chunk blocks run
                            # while the first h0 tanh is on the scalar engine
                            if dc == 0 and marshal_h1_pending[0]:
                                marshal_h1_pending[0] = False
                                marshal(1)
                            for i in pending_h1.pop(dc, []):
                                emit_parta_one(dc, i, 1)
                        if bi is not None:
                            pc = pc_tiles[dc][h]
                            for sub in range(NSUB):
                                nc.tensor.matmul(
                                    pc[:, sub * 512:(sub + 1) * 512],
                                    wbts[gi],
                                    v[gc][h][:, sub * 512:(sub + 1) * 512],
                                    start=False, stop=(gi == last_gi),
                                    skip_group_check=True)
                    for h in range(2):
                        # full-chunk tanh: earlier levels recompute identical
                        # values, later rows get overwritten before any read
                        nc.scalar.activation(
                            v[gc][h][:, :], pc_tiles[dc][h][:, :],
                            mybir.ActivationFunctionType.Tanh,
                            bias=bias_sb[:, dc:dc + 1],
                            scale=resp_sb[:, dc:dc + 1])
                    for _ in range(per):
                        if nxt:
                            emit_parta_one(dc + 1, nxt.pop(0), 0)
                # extract this chunk's output rows: transpose back to
                # batch-major and convert (rides inside the cascade except
                # for the final chunk)
                rgs = [(r0, ln, c0) for c, r0, ln, c0 in out_ranges if c == dc]
                if rgs:
                    for h in range(2):
                        po = mtp.tile([128, NBT // 2, 128], F16, tag="mt",
                                      name=f"po{dc}h{h}")
                        for q in range(NBT // 2):
                            nc.tensor.transpose(
                                po[:, q, :],
                                v[gc][h][:, q * 128:(q + 1) * 128],
                                ident[:])
                        for r0, ln, c0 in rgs:
                            nc.vector.tensor_copy(
                                ob[:, h * (NBT // 2):(h + 1) * (NBT // 2),
                                   c0:c0 + ln],
                                po[:, :, r0:r0 + ln])
                # h0 stragglers right after this chunk's last h0 ACT; h1
                # blocks are deferred into the next chunk's first level
                for i in nxt + nxt_late:
                    emit_parta_one(dc + 1, i, 0)
                if dc + 1 < n_node_chunks:
                    pending_h1[dc + 1] = list(range(len(parta[dc + 1])))

            for q in range(4):
                eng = nc.sync if q % 2 == 0 else nc.scalar
                eng.dma_start(orr[:, q * 4:(q + 1) * 4, :],
                              ob[:, q * 4:(q + 1) * 4, :])

    nc.compile()
    return nc


_CACHE = {}


def _get_compiled(key, plan):
    if key not in _CACHE:
        _CACHE[key] = _build_nc(plan)
    return _CACHE[key]


def kernel(inputs, edge_w, bias, response, in_idx, edge_mask, out_idx, output_idx):
    inputs = np.ascontiguousarray(np.asarray(inputs, dtype=np.float32))
    plan = _plan(in_idx, edge_mask, edge_w, bias, response, out_idx, output_idx)

    key = (plan["wa"].tobytes(), plan["wb"].tobytes(), str(plan["out_ranges"]),
           plan["bias_c"].tobytes(), plan["resp_c"].tobytes())
    nc = _get_compiled(hash(key), plan)

    base = {
        "wa": np.ascontiguousarray(plan["wa"]),
        "wb": np.ascontiguousarray(plan["wb"]),
        "br_c": np.ascontiguousarray(
            np.concatenate([plan["bias_c"], plan["resp_c"]], axis=1)),
    }
    if len(base["wa"]) == 0:
        base["wa"] = np.zeros((1, 128, 128), np.float16)
    if len(base["wb"]) == 0:
        base["wb"] = np.zeros((1, 128, 128), np.float16)

    x16 = inputs.astype(np.float16)
    in_maps = []
    for c in range(NCORES):
        m = dict(base)
        m["x"] = np.ascontiguousarray(x16[c * BC:(c + 1) * BC])
        in_maps.append(m)

    res = run_bass_kernel_spmd(nc, in_maps, core_ids=list(range(NCORES)))
    kernel.last_results = res
    colmap = np.asarray(plan["colmap"])
    out = np.concatenate(
        [res.results[c]["o"][:, colmap] for c in range(NCORES)], axis=0)
    return np.ascontiguousarray(out.astype(np.float32))


kernel.last_results = None

